# revision 35
# baseline (speedup 1.0000x reference)
"""Multi-head causal attention (B=1, S=4096, D=1024, H=16, HD=64) on 8
Trainium2 NeuronCores.

Sharding: head-parallel — 16 heads / 8 cores = 2 heads per core (one
128-channel slice of the QKV/output projections per core).

Per-core algorithm (all layouts transposed so the contraction dim sits on
SBUF partitions and softmax exp reads PSUM directly):
  phase 1  QKV projections from pre-transposed xT [D, S]:
             qT, kT [128, 4096] (d-contract matmuls, outputs transposed)
             V natural [4096, 128] via one extra PE transpose per 128-tile,
             stored interleaved with ones columns: [V_A | 1 | V_B | 1]
  phase 2  flash-style attention, no max-subtraction (scores ~ N(0,1)):
             scoresT psum [j, q] = kT_j.T @ qT_q  (2 heads packed via
             tile_position row strips, K=64 each)
             PT = exp(scoresT/8)  (ScalarE, reads PSUM, writes SBUF f32r)
             causal: strictly-upper j-blocks skipped, 4 diagonal mask
             tiles multiplied post-exp
             acc[65, q] += [V_j | 1].T @ PT_j  (M=65: row 64 = softmax
             denominator l for free)
             normalize: attnT[hd, q] = acc[0:64] * (1/l) (gpsimd
             partition-broadcast + DVE mul)
  phase 3  output projection partial: partialT[o, s] = WoT_c.T @ attnT,
             written transposed [1024, 4096] per core.

Host: sums the 8 partials and transposes back to [1, S, D].

Matmuls run in float32r (TF32-like, ~1.5e-4 rel err per matmul, 1 cyc/row
at N>=256 vs 4 cyc/row for plain fp32).
"""

import os
import sys

import numpy as np

for _p in ("/opt/trn_rl_repo", "/root/.axon_site/_ro/trn_rl_repo"):
    if os.path.isdir(_p) and _p not in sys.path:
        sys.path.insert(0, _p)

from contextlib import ExitStack

import concourse.bass as bass
import concourse.tile as tile
from concourse import bacc, bass_utils, mybir
from concourse.masks import make_identity
import concourse.hw_specs as _hw_specs
import functools as _functools

# Pin Exp/Ln to the one table set containing both, so the softmax exp and
# the exp(-ln(l)) normalization never ping-pong ACT_TABLE_LOADs. Only the
# *selection* map is filtered; set order (= act_func_set_id) is unchanged.
_orig_get_tables = _hw_specs.get_activation_tables


@_functools.cache
def _pinned_tables(arch):
    t = dict(_orig_get_tables(arch))
    strip = {mybir.ActivationFunctionType.Exp, mybir.ActivationFunctionType.Ln}
    for name in t:
        if name != "natural_log_exp_and_others":
            t[name] = t[name] - strip
    return t


_hw_specs.get_activation_tables = _pinned_tables
bacc.get_activation_tables = _pinned_tables

# Problem shape (hardcoded per the harness contract).
B, S, D, H = 1, 4096, 1024, 16
HD = D // H          # 64
NCORES = 8
HPC = H // NCORES    # 2 heads per core
M = HPC * HD         # 128 channels per core
SBK = 512            # s/q block size
NSB = S // SBK       # 8
DBK = 128            # d block size
NDB = D // DBK       # 8
JBK = 128            # j (key) block size
GJ = 2               # j-blocks per exp group ([128, 1024] psum = 2 banks)
VW = 3 * HD          # v_aug row width per j-tile: [V_A | V_B | ones]

F32 = mybir.dt.float32
F32R = mybir.dt.float32r

_CACHE = {}


def _build_nc():
    """Build + compile the per-core Bass program (identical on all cores)."""
    nc = bacc.Bacc("TRN2", target_bir_lowering=False, debug=False,
                   num_devices=NCORES)

    xT = nc.dram_tensor("xT", [D, S], F32R, kind="ExternalInput").ap()
    wq = nc.dram_tensor("wq", [D, M], F32R, kind="ExternalInput").ap()
    wk = nc.dram_tensor("wk", [D, M], F32R, kind="ExternalInput").ap()
    wv = nc.dram_tensor("wv", [D, M], F32R, kind="ExternalInput").ap()
    wo = nc.dram_tensor("wo", [M, D], F32R, kind="ExternalInput").ap()
    ones = nc.dram_tensor("ones", [128, HD], F32R, kind="ExternalInput").ap()
    dmask = nc.dram_tensor("dmask", [JBK, JBK], F32R,
                           kind="ExternalInput").ap()
    outp = nc.dram_tensor("outp", [D, S], F32, kind="ExternalOutput").ap()

    with tile.TileContext(nc) as tc:
        with ExitStack() as ctx:
            _emit(ctx, tc, nc, xT, wq, wk, wv, wo, ones, dmask, outp)
    nc.compile()
    return nc


def _emit(ctx, tc, nc, xT, wq, wk, wv, wo, ones, dmask, outp):
    const = ctx.enter_context(tc.tile_pool(name="const", bufs=1))
    persist = ctx.enter_context(tc.tile_pool(name="persist", bufs=1))
    xt_pool = ctx.enter_context(tc.tile_pool(name="xt", bufs=12))
    vtmp_pool = ctx.enter_context(tc.tile_pool(name="vtmp", bufs=2))
    pt_pool = ctx.enter_context(tc.tile_pool(name="pt", bufs=8))
    out_pool = ctx.enter_context(tc.tile_pool(name="outt", bufs=3))
    small = ctx.enter_context(tc.tile_pool(name="small", bufs=4))
    ps6k = ctx.enter_context(tc.tile_pool(name="ps6k", bufs=2, space="PSUM"))
    ps2k = ctx.enter_context(tc.tile_pool(name="ps2k", bufs=4, space="PSUM"))

    # ---- constants / persistent SBUF ----
    ident = const.tile([128, 128], F32)
    make_identity(nc, ident)

    wq_sb = const.tile([128, D], F32R)   # 8 d-tiles side by side [d, m]
    wk_sb = const.tile([128, D], F32R)
    wv_sb = const.tile([128, D], F32R)
    wo_sb = const.tile([128, D], F32R)   # [m, o]
    def _w_chunk(w_sb, w_dram, c, n=2):
        w_r = w_dram.rearrange("(d p) m -> p d m", p=DBK)
        w_sb_r = w_sb[:].rearrange("p (d m) -> p d m", d=NDB)
        nc.sync.dma_start(out=w_sb_r[:, c:c + n, :], in_=w_r[:, c:c + n, :])

    wtrips = ((wq_sb, wq), (wk_sb, wk), (wv_sb, wv))
    for c in (0, 2, 4, 6):
        for w_sb, w_dram in wtrips:
            _w_chunk(w_sb, w_dram, c, 2)

    def _late_consts():
        for c in range(4):
            nc.sync.dma_start(out=wo_sb[:, bass.ts(c, 256)],
                              in_=wo[:, bass.ts(c, 256)])
        nc.sync.dma_start(out=mask_sb[:], in_=dmask[:])

    mask_sb = const.tile([128, JBK], F32R)

    qT_sb = persist.tile([128, S], F32R)
    kT_sb = persist.tile([128, S], F32R)
    NJT = S // JBK   # 32 j-tiles
    v_aug = persist.tile([128, NJT * VW], F32R)
    attnT = persist.tile([128, S], F32R)

    # per j-tile layout [V_A | ones | V_B]: head A reads cols 0:128
    # ([V_A | 1] -> acc rows 64:128 = denominator l), head B reads cols
    # 64:192 ([1 | V_B] -> acc rows 0:64 = l). ones filled by one
    # broadcast DMA (step-0 middle dim on input).
    v_aug_r = v_aug[:].rearrange("p (t c w) -> p t c w", c=3, w=HD)
    ones_bcast = bass.AP(
        tensor=ones.tensor, offset=0,
        ap=[[HD, 128], [0, NJT], [1, HD]])
    nc.sync.dma_start(out=v_aug_r[:, :, 1, :], in_=ones_bcast)

    def phase1(sb):
        """QKV projections for s-block sb (512 rows of the sequence)."""
        xts = []
        for d in range(NDB):
            xt = xt_pool.tile([128, SBK], F32R, name=f"xt_{sb}_{d}", tag="xt")
            if sb == 0:
                half = SBK // 2
                for h in range(2):
                    nc.gpsimd.dma_start(
                        out=xt[:, h * half:(h + 1) * half],
                        in_=xT[bass.ts(d, DBK),
                               sb * SBK + h * half:sb * SBK + (h + 1) * half])
            else:
                nc.gpsimd.dma_start(out=xt[:],
                                    in_=xT[bass.ts(d, DBK), bass.ts(sb, SBK)])
            xts.append(xt)
        # sequential short chains: each psum slot is held ~8 matmuls, not 24
        k_ps = ps6k.tile([128, SBK], F32, tag="sc")
        for d in range(NDB):
            nc.tensor.matmul(k_ps[:], lhsT=wk_sb[:, bass.ts(d, M)],
                             rhs=xts[d][:], start=d == 0, stop=d == NDB - 1)
        nc.vector.tensor_copy(kT_sb[:, bass.ts(sb, SBK)], k_ps[:])
        q_ps = ps6k.tile([128, SBK], F32, tag="sc")
        for d in range(NDB):
            nc.tensor.matmul(q_ps[:], lhsT=wq_sb[:, bass.ts(d, M)],
                             rhs=xts[d][:], start=d == 0, stop=d == NDB - 1)
        nc.vector.tensor_copy(qT_sb[:, bass.ts(sb, SBK)], q_ps[:])
        vT_ps = ps2k.tile([128, SBK], F32, tag="small")
        for d in range(NDB):
            nc.tensor.matmul(vT_ps[:], lhsT=wv_sb[:, bass.ts(d, M)],
                             rhs=xts[d][:], start=d == 0, stop=d == NDB - 1)
        vt = vtmp_pool.tile([128, SBK], F32)
        nc.vector.tensor_copy(vt[:], vT_ps[:])
        # vT [m, s] -> V natural [s, m] per 128-tile, into v_aug slots
        for t in range(SBK // JBK):
            jt = sb * (SBK // JBK) + t     # global j-tile index
            tp_ps = ps2k.tile([128, 128], F32, tag="small")
            nc.tensor.transpose(tp_ps[:], vt[:, bass.ts(t, 128)], ident[:])
            nc.vector.tensor_copy(v_aug_r[:, jt, 0::2, :], tp_ps[:, 0:2 * HD])

    def attention(qb):
        """Causal attention for query block qb (both heads)."""
        nj = 4 * (qb + 1)               # valid j128-blocks
        acc_A = ps2k.tile([128, SBK], F32, tag="small")
        acc_B = ps2k.tile([128, SBK], F32, tag="small")
        qsl = bass.ts(qb, SBK)
        for j in range(nj):
            # diagonal j-block r: columns q < 128*r are fully masked ->
            # process only [off:SBK] (partial moving operand).
            # sc/pt hold BOTH heads ([A | B]) so the row-packed score
            # matmul pair shares one slot release and issues back-to-back.
            r = j - (nj - 4)
            off = 128 * r if r > 0 else 0
            sc = ps6k.tile([128, 2 * SBK], F32, tag="sc")
            qa = qT_sb[0:64, qb * SBK + off:(qb + 1) * SBK]
            qb_ap = qT_sb[64:128, qb * SBK + off:(qb + 1) * SBK]
            nc.tensor.matmul(sc[:, off:SBK],
                             lhsT=kT_sb[0:64, bass.ts(j, JBK)],
                             rhs=qa, start=True, stop=True)
            nc.tensor.matmul(sc[:, SBK + off:2 * SBK],
                             lhsT=kT_sb[64:128, bass.ts(j, JBK)],
                             rhs=qb_ap, start=True, stop=True)
            pt = pt_pool.tile([128, 2 * SBK], F32R, tag="pt")
            scale = float(1.0 / np.sqrt(HD))
            if off == 0:
                nc.scalar.activation(pt[:], sc[:],
                                     mybir.ActivationFunctionType.Exp,
                                     scale=scale)
            else:
                w = SBK - off
                sc2 = bass.AP(tensor=sc.tensor, offset=sc.offset + off,
                              ap=[list(sc.ap[0]), [SBK, 2], [1, w]])
                pt2 = bass.AP(tensor=pt.tensor, offset=pt.offset + off,
                              ap=[list(pt.ap[0]), [SBK, 2], [1, w]])
                nc.scalar.activation(pt2, sc2,
                                     mybir.ActivationFunctionType.Exp,
                                     scale=scale)
            if r >= 0:
                # triangle mask on the [128,128] diagonal strip, per head
                dlo = 128 * r
                nc.gpsimd.tensor_mul(pt[:, dlo:dlo + 128],
                                     pt[:, dlo:dlo + 128], mask_sb[:])
                nc.gpsimd.tensor_mul(pt[:, SBK + dlo:SBK + dlo + 128],
                                     pt[:, SBK + dlo:SBK + dlo + 128],
                                     mask_sb[:])
            st, sp = j == 0, j == nj - 1
            vb = j * VW
            nc.tensor.matmul(acc_A[:, off:SBK],
                             lhsT=v_aug[:, vb:vb + 128],
                             rhs=pt[:, off:SBK], start=st, stop=sp)
            nc.tensor.matmul(acc_B[:, off:SBK],
                             lhsT=v_aug[:, vb + HD:vb + VW],
                             rhs=pt[:, SBK + off:2 * SBK],
                             start=st, stop=sp)
        return acc_A, acc_B

    def normalize(qb, acc_A, acc_B):
        # head A: out rows 0:64, l rows 64:128; head B flipped
        qsl = bass.ts(qb, SBK)
        # 1/l as exp(-ln(l)) on ScalarE (natural_log_exp_and_others set
        # holds both funcs, so no table switches).
        lnl = small.tile([64, SBK], F32, tag="lnl")
        nc.scalar.activation(lnl[:], acc_A[HD:2 * HD, :],
                             mybir.ActivationFunctionType.Ln)
        linv = small.tile([64, SBK], F32, tag="linv")
        nc.scalar.activation(linv[:], lnl[:],
                             mybir.ActivationFunctionType.Exp, scale=-1.0)
        nc.vector.tensor_mul(attnT[0:64, qsl], acc_A[0:HD, :], linv[:])
        lnl_b = small.tile([64, SBK], F32, tag="lnl")
        nc.scalar.activation(lnl_b[:], acc_B[0:HD, :],
                             mybir.ActivationFunctionType.Ln)
        linv_b = small.tile([64, SBK], F32, tag="linv")
        nc.scalar.activation(linv_b[:], lnl_b[:],
                             mybir.ActivationFunctionType.Exp, scale=-1.0)
        nc.vector.tensor_mul(attnT[64:128, qsl], acc_B[HD:2 * HD, :],
                             linv_b[:])

    def proj(qb):
        """Output-projection partial for s-block qb -> DRAM (transposed)."""
        qsl = bass.ts(qb, SBK)
        for ob in range(NDB):
            po = ps2k.tile([128, SBK], F32, tag="small")
            nc.tensor.matmul(po[:], lhsT=wo_sb[:, bass.ts(ob, 128)],
                             rhs=attnT[:, qsl], start=True, stop=True)
            ot = out_pool.tile([128, SBK], F32)
            nc.vector.tensor_copy(ot[:], po[:])
            nc.sync.dma_start(out=outp[bass.ts(ob, 128), qsl], in_=ot[:])

    # interleaved emission: attention(qb) only needs kT/v for s-blocks <= qb.
    # phase1(sb+1) is emitted before normalize(qb) so its PSUM evictions are
    # not queued behind the slow DVE reciprocals.
    phase1(0)
    _late_consts()
    accs = None
    for sb in range(1, NSB):
        if accs is not None:
            normalize(sb - 2, *accs)
            proj(sb - 2)
        accs = attention(sb - 1)
        phase1(sb)
    normalize(NSB - 2, *accs)
    proj(NSB - 2)
    accs = attention(NSB - 1)
    normalize(NSB - 1, *accs)
    proj(NSB - 1)


def _host_prep(x, Wq, Wk, Wv, Wo):
    xT = np.ascontiguousarray(x.reshape(S, D).T).astype(np.float32)
    jj = np.arange(JBK)[:, None]
    qq = np.arange(JBK)[None, :]
    dmask = (jj <= qq).astype(np.float32)
    in_maps = []
    for c in range(NCORES):
        sl = slice(c * M, (c + 1) * M)
        in_maps.append({
            "xT": xT,
            "wq": np.ascontiguousarray(Wq[sl, :].T).astype(np.float32),
            "wk": np.ascontiguousarray(Wk[sl, :].T).astype(np.float32),
            "wv": np.ascontiguousarray(Wv[sl, :].T).astype(np.float32),
            "wo": np.ascontiguousarray(Wo[:, sl].T).astype(np.float32),
            "ones": np.ones((128, HD), dtype=np.float32),
            "dmask": dmask,
        })
    return in_maps


def _run(inputs, trace=False):
    x = np.asarray(inputs["x"], dtype=np.float32)
    Wq = np.asarray(inputs["Wq"], dtype=np.float32)
    Wk = np.asarray(inputs["Wk"], dtype=np.float32)
    Wv = np.asarray(inputs["Wv"], dtype=np.float32)
    Wo = np.asarray(inputs["Wo"], dtype=np.float32)

    if "nc" not in _CACHE:
        _CACHE["nc"] = _build_nc()
    nc = _CACHE["nc"]

    in_maps = _host_prep(x, Wq, Wk, Wv, Wo)
    res = bass_utils.run_bass_kernel_spmd(
        nc, in_maps, core_ids=list(range(NCORES)), trace=trace)
    partial = np.zeros((D, S), dtype=np.float64)
    for c in range(NCORES):
        partial += res.results[c]["outp"].astype(np.float64)
    out = partial.T.astype(np.float32).reshape(B, S, D)
    return out, res


def kernel(x, mask, Wq, Wk, Wv, Wo):
    mask = np.asarray(mask)
    causal = np.tril(np.ones((S, S), dtype=bool))
    if mask.reshape(S, S).shape == causal.shape and bool(
            np.array_equal(mask.reshape(S, S), causal)):
        out, _ = _run({"x": x, "Wq": Wq, "Wk": Wk, "Wv": Wv, "Wo": Wo})
        return out
    # safety net for a non-causal mask: exact numpy fallback
    return _numpy_ref(np.asarray(x, np.float32), mask,
                      np.asarray(Wq, np.float32), np.asarray(Wk, np.float32),
                      np.asarray(Wv, np.float32), np.asarray(Wo, np.float32))


def _numpy_ref(x, mask, Wq, Wk, Wv, Wo):
    q = (x @ Wq.T).reshape(B, S, H, HD).transpose(0, 2, 1, 3)
    k = (x @ Wk.T).reshape(B, S, H, HD).transpose(0, 2, 1, 3)
    v = (x @ Wv.T).reshape(B, S, H, HD).transpose(0, 2, 1, 3)
    sc = np.einsum("bhqd,bhkd->bhqk", q, k) / np.sqrt(np.float32(HD))
    sc = np.where(mask.reshape(1, 1, S, S), sc, -1e9)
    sc = sc - sc.max(axis=-1, keepdims=True)
    p = np.exp(sc)
    p = p / p.sum(axis=-1, keepdims=True)
    o = np.einsum("bhqk,bhkd->bhqd", p, v)
    o = o.transpose(0, 2, 1, 3).reshape(B, S, D)
    return (o @ Wo.T).astype(np.float32)
